# revision 7
# baseline (speedup 1.0000x reference)
"""CRF log-prob kernel for Trainium2 (8 NeuronCores, batch-sharded).

Math: the log-semiring forward scan
    alpha_t[b,j] = e_t[b,j] + logsumexp_i(alpha_{t-1}[b,i] + T[i,j])
is computed in the exp domain as plain matmuls:
    u_t = (E^T @ u_{t-1}) * W_t,   E = exp(T),  W_t[j,b] = exp(e_t[b,j] - D_t[b])
with host-chosen per-step shifts D_t keeping u in fp32 range (the shifts
cancel exactly in the final logZ, which adds sum_t D_t back).

Per core: 32 batch columns, state u kept as [128 tags x 32 cols] in an SBUF
ring (32 slots); each step is one PE matmul (E stationary) + one DVE
elementwise multiply per 16-column group (2 groups pipeline PE against DVE).
Ragged lengths: every u_t is snapshotted to DRAM in batched 16-step DMAs and
the host reads u at t = len_b - 1 per column; steps past a column's length
multiply by a harmless decaying pad. Score (the gather part, O(B*T) work) and
the final O(B*N) readout run on host.
"""

import sys

import numpy as np

if "/opt/trn_rl_repo" not in sys.path:
    sys.path.insert(0, "/opt/trn_rl_repo")

B, T, N = 256, 512, 128
NCORES = 8
BC = B // NCORES          # batch columns per core
G = 2                     # pipeline groups per core
GC = BC // G              # columns per group
R = 32                    # u ring slots
BLK = 16                  # snapshot DMA batch (steps per DMA)
WCHUNK = 32               # scan steps per W chunk tile
C_HAT = 2.8               # expected per-step logsumexp growth beyond max_j e
PAD = 2.0 ** -10

_BUILT = {}


def _build_program():
    if "nc" in _BUILT:
        return _BUILT["nc"]

    import concourse.bacc as bacc
    import concourse.tile as tile
    from concourse import mybir

    f32 = mybir.dt.float32
    bf16 = mybir.dt.bfloat16
    nc = bacc.Bacc(None, target_bir_lowering=False, debug=False)

    e_d = nc.dram_tensor("e_mat", [N, N], bf16, kind="ExternalInput")
    u0_d = nc.dram_tensor("u0", [N, BC], bf16, kind="ExternalInput")
    # W for steps t=1..T-1, transposed per core: [j, (t-1)*BC + b]
    w_d = nc.dram_tensor("w_mat", [N, (T - 1) * BC], f32, kind="ExternalInput")
    snap_d = nc.dram_tensor("snap", [N, T, BC], bf16, kind="ExternalOutput")

    nchunks = (T - 1 + WCHUNK - 1) // WCHUNK  # 16 chunks: last covers 31 steps

    with tile.TileContext(nc) as tc:
        with (
            tc.tile_pool(name="const", bufs=1) as constp,
            tc.tile_pool(name="psA", bufs=4, space="PSUM") as psa,
            tc.tile_pool(name="psB", bufs=4, space="PSUM") as psb,
        ):
            e_sb = constp.tile([N, N], bf16, tag="e_sb")
            nc.sync.dma_start(e_sb[:], e_d[:])

            ring = constp.tile([N, R * BC], bf16, tag="ring")
            nc.sync.dma_start(ring[:, 0:BC], u0_d[:])

            wtiles = []
            for k in range(nchunks):
                c0 = k * WCHUNK * BC
                c1 = min((T - 1) * BC, (k + 1) * WCHUNK * BC)
                wt = constp.tile([N, c1 - c0], f32, tag=f"w{k}")
                nc.sync.dma_start(wt[:], w_d[:, c0:c1])
                wtiles.append(wt)

            for t in range(1, T):
                src0 = ((t - 1) % R) * BC
                dst0 = (t % R) * BC
                wt = wtiles[(t - 1) // WCHUNK]
                woff = ((t - 1) % WCHUNK) * BC
                for g, psp in ((0, psa), (1, psb)):
                    lo = g * GC
                    ps = psp.tile([N, GC], f32, tag=f"ps{g}")
                    nc.tensor.matmul(
                        ps[:],
                        e_sb[:],
                        ring[:, src0 + lo : src0 + lo + GC],
                        start=True,
                        stop=True,
                    )
                    nc.vector.tensor_tensor(
                        ring[:, dst0 + lo : dst0 + lo + GC],
                        ps[:],
                        wt[:, woff + lo : woff + lo + GC],
                        mybir.AluOpType.mult,
                    )
                if t % BLK == BLK - 1 or t == T - 1:
                    # snapshot the just-completed aligned block of BLK slots
                    tb = (t // BLK) * BLK
                    s0 = (tb % R) * BC
                    nc.gpsimd.dma_start(
                        snap_d[:, tb : tb + BLK, :],
                        ring[:, s0 : s0 + BLK * BC],
                    )

    if not nc.is_finalized():
        nc.finalize()
    _BUILT["nc"] = nc
    return nc


def _host_prep(log_potentials, transition, start_transition, end_transition, lengths):
    lp = np.asarray(log_potentials, np.float32)
    trans = np.asarray(transition, np.float32)
    start = np.asarray(start_transition, np.float32)
    lengths = np.asarray(lengths, np.int64)

    D = np.empty((B, T), np.float32)
    D[:, 0] = (start[None, :] + lp[:, 0, :]).max(axis=1)
    D[:, 1:] = lp[:, 1:, :].max(axis=2) + C_HAT

    import ml_dtypes

    e_mat = np.exp(trans).astype(ml_dtypes.bfloat16)

    active = np.arange(T)[None, :] < lengths[:, None]          # [B,T]
    W = np.exp(lp - D[:, :, None])
    W = np.where(active[:, :, None], W, PAD).astype(np.float32)  # [B,T,N]

    u0 = np.exp(start[None, :] + lp[:, 0, :] - D[:, 0, None]).astype(np.float32)  # [B,N]

    in_maps = []
    for c in range(NCORES):
        bs = slice(c * BC, (c + 1) * BC)
        # [N, T-1, BC] -> [j, (t-1)*BC + b] (t-major, b-minor) for steps 1..T-1
        w_core = np.ascontiguousarray(
            W[bs, 1:, :].transpose(2, 1, 0).reshape(N, (T - 1) * BC)
        )
        in_maps.append(
            {
                "e_mat": e_mat,
                "u0": np.ascontiguousarray(u0[bs].T).astype(ml_dtypes.bfloat16),
                "w_mat": w_core,
            }
        )
    return in_maps, D


def _host_score(lp, trans, start, end, target, lengths):
    tidx = np.arange(T)
    valid = tidx[None, :] < lengths[:, None]
    emis = np.take_along_axis(lp, target[..., None], axis=-1)[..., 0]
    emis_score = np.where(valid, emis, 0.0).sum(axis=1, dtype=np.float64)
    tr = trans[target[:, :-1], target[:, 1:]]
    tr_score = np.where(valid[:, 1:], tr, 0.0).sum(axis=1, dtype=np.float64)
    last = target[np.arange(B), lengths - 1]
    return emis_score + tr_score + start[target[:, 0]] + end[last]


def kernel(log_potentials, transition, start_transition, end_transition, target, lengths):
    from concourse.bass_utils import run_bass_kernel_spmd

    out_dtype = np.asarray(log_potentials).dtype
    lp = np.asarray(log_potentials, np.float32)
    trans = np.asarray(transition, np.float32)
    start = np.asarray(start_transition, np.float32)
    end = np.asarray(end_transition, np.float32)
    target_i = np.asarray(target).astype(np.int64)
    lengths_i = np.asarray(lengths).astype(np.int64)

    nc = _build_program()
    in_maps, D = _host_prep(lp, trans, start, end, lengths_i)
    res = run_bass_kernel_spmd(nc, in_maps, list(range(NCORES)))
    results = res.results

    # ---- host readout ----
    expE = np.exp(end).astype(np.float64)
    logZ = np.empty(B, np.float64)
    for c in range(NCORES):
        snap = results[c]["snap"]                      # [N, T, BC]
        for p in range(BC):
            b = c * BC + p
            L = int(lengths_i[b])
            u = snap[:, L - 1, p].astype(np.float64)
            r = float((u * expE).sum())
            logZ[b] = np.log(r) + D[b, :L].sum(dtype=np.float64)

    score = _host_score(lp, trans, start, end, target_i, lengths_i)
    return (score - logZ).astype(out_dtype if out_dtype in (np.float32, np.float64) else np.float32)
